# revision 1
# baseline (speedup 1.0000x reference)
"""Trainium2 Bass kernel for nn_MCQuantiles (ThreeCompNode SNN scan).

Strategy (8 NeuronCores, data-parallel over batch):
- Each core takes 8 batches x 32 samples = 256 rows of the B*S axis.
- Everything runs in "transposed space": feature dims on SBUF partitions,
  batch-rows on the free dim. All transposes/swizzles are done host-side for
  free; every DMA is a flat contiguous [128, X] block.
- The input matmuls (te @ Wa.T, se @ Wb.T) don't depend on the recurrence, so
  apical is computed for pairs of time steps with N=512 moving operands.
- Membrane recurrences use 2^t-scaled state so each update is a single fused
  scalar_tensor_tensor op reading the matmul result straight from PSUM:
      alpha_t = alpha_{t-1} + 2^t * apical_t         (alpha = 2^{t+1} ma)
      mu_t    = mu_{t-1} + 0.5*alpha_t + 0.5*beta_t  (mu = 2^{t+1} ms)
      spike   <=> mu > 2^{t+1}
- Layer-1 spikes are fed to the W1 matmul as q = NOT(spike) with the
  rowsum(W1)+b1 constant folded in host-side (h = c1 - q @ W1.T).
- Layer-2 spikes sp2 are fed directly to the W2 matmul; out accumulates in a
  persistent PSUM bank over all T, evicted once with scale 1/T + bias b2.
- Matmuls run in bf16 (full PE rate). Binary spike inputs are bf16-exact; the
  LIF threshold margin (|ml|max ~0.35 vs th 0.5) makes output spikes immune to
  bf16 rounding of the weights.
"""
import numpy as np
import ml_dtypes

import bass_rust
import concourse.bass as bass
import concourse.mybir as mybir
from concourse.bass_utils import run_bass_kernel_spmd
from concourse.tile import TileContext
from concourse.tile_rust import add_dep_helper

# ----- problem constants (hardcoded per contract) -----
T, B, S = 8, 64, 32
DS = DT = 3136
F = H = 512
L = 18
N_CORES = 8
NB = B // N_CORES              # 8 batches per core
R = NB * S                     # 256 rows per core
KD = 3200                      # 3136 padded to 25 k-tiles of 128
NK = KD // 128                 # 25
NPAIR = T // 2                 # 4 step pairs
NG = F // 128                  # 4 f-tiles (= h-tiles)

# column offsets inside the bf16 weight walls [128, *]
WA_COLS = NK * F               # wallA: apical weights only
O_WB = 0                       # wallM: basal weights, NK*F cols
O_SE = O_WB + NK * F           # state embeddings, NK*T*NB cols
WM_COLS = O_SE + NK * T * NB
O_W1 = 0                       # wallB: W1.T, NG*H cols
O_W2 = O_W1 + NG * H           # W2.T, NG*L cols
WB_COLS = O_W2 + NG * L

F32 = mybir.dt.float32
BF16 = mybir.dt.bfloat16
OP = mybir.AluOpType


def _patch_tile_drain():
    """This walrus build allows a single sync-wait per TPB_CTRL Drain; Tile's
    kernel-tail drain attaches one wait per active logical proc. Split them
    across a chain of drains."""
    def _patched(self, tick_clock, wait_clock):
        nc = self.nc
        drain_inst = nc.sync.drain()
        wait_clock.add_sem_waits(
            drain_inst.ins, bass_rust.ScopedClock({None: tick_clock.global_clock})
        )
        si = drain_inst.ins.sync_info
        if si is not None and len(si.on_wait) > 1:
            waits = list(si.on_wait)
            drain_inst.ins.sync_info = mybir.SyncInfo(
                on_wait=waits[:1], on_update=list(si.on_update)
            )
            for w in waits[1:]:
                extra = nc.sync.drain()
                extra.ins.sync_info = mybir.SyncInfo(on_wait=[w], on_update=[])
        nc.all_engine_barrier()
        popped = nc._tile_sem_poison_stack.pop()
        assert popped is self._sem_poison
        nc.clear_and_free_semaphores(list(self.sems.allocated().values()))
        nc.all_engine_barrier()

    TileContext._drain_and_barrier = _patched


def _split_excess_waits(nc, limit=1):
    """Walrus here rejects instructions carrying more than ~1 sync-wait. Move
    excess waits onto same-engine NoOps inserted just before the instruction."""
    for fn in nc.m.functions:
        for bb in fn.blocks:
            new = []
            changed = False
            for inst in bb.instructions:
                si = getattr(inst, "sync_info", None)
                ow = list(si.on_wait) if si is not None and si.on_wait else []
                if len(ow) > limit:
                    extra = ow[limit:]
                    for j in range(0, len(extra), limit):
                        nop = mybir.InstNoOp(
                            name=f"{inst.name}-ws{j}", ins=[], outs=[]
                        )
                        nop.engine = inst.engine
                        nop.sync_info = mybir.SyncInfo(
                            on_wait=extra[j : j + limit], on_update=[]
                        )
                        new.append(nop)
                    inst.sync_info = mybir.SyncInfo(
                        on_wait=ow[:limit], on_update=list(si.on_update)
                    )
                    changed = True
                new.append(inst)
            if changed:
                bb.set_instructions(new) if hasattr(bb, "set_instructions") else None
                if not hasattr(bb, "set_instructions"):
                    try:
                        bb.instructions[:] = new
                    except TypeError:
                        bb.instructions = new


def build_nc(with_b1=False, state_dt=BF16):  # with_b1 unused
    _patch_tile_drain()
    nc = bass.Bass()

    teT = nc.declare_dram_parameter("teT", [NPAIR, 128, NK * 2 * R], BF16, isOutput=False)
    wallA = nc.declare_dram_parameter("wallA", [128, WA_COLS], BF16, isOutput=False)
    wallM = nc.declare_dram_parameter("wallM", [128, WM_COLS], BF16, isOutput=False)
    wallB = nc.declare_dram_parameter("wallB", [128, WB_COLS], BF16, isOutput=False)
    cons = nc.declare_dram_parameter("cons", [128, NG * T + 1 + 2 * T], F32, isOutput=False)
    out = nc.declare_dram_parameter("out", [L, R], F32, isOutput=True)

    with TileContext(nc) as tc:
        with (
            tc.tile_pool(name="wpool", bufs=1) as wpool,
            tc.tile_pool(name="tepool", bufs=2) as tepool,
            tc.tile_pool(name="state", bufs=1) as state,
            tc.tile_pool(name="qpool", bufs=2) as qpool,
            tc.tile_pool(name="appool", bufs=4, space="PSUM") as appool,
            tc.tile_pool(name="hpool", bufs=3, space="PSUM") as hpool,
            tc.tile_pool(name="opool", bufs=1, space="PSUM") as opool,
        ):
            # ---- resident weights/constants ----
            CHUNKS = [2, 3, 5, 5, 5, 5]   # k-tiles per DMA chunk (25 total)
            CH_OFF = [0, 2, 5, 10, 15, 20]
            NCH = 5
            NCHUNK = len(CHUNKS)
            wallA_c = []
            prev_wa_dma = None
            for c in range(NCHUNK):
                wa_ck = wpool.tile(
                    [128, CHUNKS[c] * F], BF16, tag=f"wallA{c}", name=f"wa_ck{c}"
                )
                wallA_c.append(wa_ck)
                d = nc.sync.dma_start(
                    wa_ck[:],
                    wallA[:, CH_OFF[c] * F : (CH_OFF[c] + CHUNKS[c]) * F],
                )
                if prev_wa_dma is not None:
                    add_dep_helper(d.ins, prev_wa_dma.ins,
                                   reason="serialize wallA chunk DMAs")
                prev_wa_dma = d
            # wallM (basal+state weights) chained after wallA so basal can
            # start mid-pair-0; wallB (W1/W2) + cons right after.
            wallM_sb = wpool.tile([128, WM_COLS], BF16, tag="wallM", name="wallM_sb")
            wallB_sb = wpool.tile([128, WB_COLS], BF16, tag="wallB", name="wallB_sb")
            cons_sb = wpool.tile([128, NG * T + 1 + 2 * T], F32, tag="cons", name="cons_sb")

            def waT(k, g):
                for c in range(NCHUNK - 1, -1, -1):
                    if k >= CH_OFF[c]:
                        kk = k - CH_OFF[c]
                        return wallA_c[c][:, kk * F + g * 128 : kk * F + (g + 1) * 128]

            def wbT(k, g):
                return wallM_sb[:, O_WB + k * F + g * 128 : O_WB + k * F + (g + 1) * 128]

            def seT(k):
                return wallM_sb[:, O_SE + k * T * NB : O_SE + (k + 1) * T * NB]

            def w1T(k, g):
                return wallB_sb[:, O_W1 + k * H + g * 128 : O_W1 + k * H + (g + 1) * 128]

            def w2T(k):
                return wallB_sb[:, O_W2 + k * L : O_W2 + (k + 1) * L]

            def c1s_ap(g, t):
                return cons_sb[:, g * T + t : g * T + t + 1]

            b2_ap = cons_sb[0:L, NG * T : NG * T + 1]

            def th1_ap(t):  # -(2^{t+1})
                c = NG * T + 1 + t
                return cons_sb[:, c : c + 1]

            def th2_ap(t):  # -(2^t)
                c = NG * T + 1 + T + t
                return cons_sb[:, c : c + 1]

            # ---- state tiles ----
            A = [[state.tile([128, R], state_dt, tag=f"A{g}_{p}", name=f"A{g}_{p}")
                  for p in range(2)] for g in range(NG)]
            M = [state.tile([128, R], state_dt, tag=f"M{g}", name=f"M{g}") for g in range(NG)]
            ML = [state.tile([128, R], state_dt, tag=f"ML{g}", name=f"ML{g}") for g in range(NG)]
            Bsc = [state.tile([128, T * NB], state_dt, tag=f"Bsc{g}", name=f"Bsc{g}")
                   for g in range(NG)]

            o_psum = opool.tile([L, R], F32, tag="o", name="o_psum")

            # ---- software-pipelined main loop ----
            # Emit order interleaves pair p's recurrent chain with pair p+1's
            # apical matmul chunks so the in-order PE never head-of-line
            # blocks on spike results from the DVE.
            def emit_te_dma(pair, chain):
                tiles = []
                prev = None
                for c in range(NCHUNK):
                    tck = tepool.tile(
                        [128, CHUNKS[c] * 2 * R], BF16, tag=f"te{c}", name=f"te_ck{c}"
                    )
                    tiles.append(tck)
                    d = nc.sync.dma_start(
                        tck[:],
                        teT[pair][:, CH_OFF[c] * 2 * R
                                  : (CH_OFF[c] + CHUNKS[c]) * 2 * R],
                    )
                    if prev is not None and chain:
                        add_dep_helper(d.ins, prev.ins,
                                       reason="serialize startup te chunk DMAs")
                    prev = d
                return tiles, prev

            def emit_ap_chunk(psums, te_tiles, c):
                for g in range(NG):
                    for kk in range(CHUNKS[c]):
                        k = CH_OFF[c] + kk
                        nc.tensor.matmul(
                            psums[g][:],
                            lhsT=waT(k, g),
                            rhs=te_tiles[c][:, kk * 2 * R : (kk + 1) * 2 * R],
                            start=(k == 0),
                            stop=(k == NK - 1),
                        )

            def emit_a_updates(ap_psum, pair):
                for sub in range(2):
                    t = 2 * pair + sub
                    for g in range(NG):
                        apq = ap_psum[g][:, sub * R : (sub + 1) * R]
                        if t == 0:
                            nc.vector.tensor_scalar(
                                A[g][0][:], apq, 0.5, None, OP.mult
                            )
                        else:
                            nc.vector.scalar_tensor_tensor(
                                A[g][t % 2][:], apq, float(2 ** (t - 1)),
                                A[g][1 - t % 2][:], OP.mult, OP.add,
                            )

            def emit_basal():
                bs_psum = hpool.tile([128, T * NB], F32, tag="hq", name="bs_psum")
                for g in range(NG):
                    for k in range(NK):
                        nc.tensor.matmul(
                            bs_psum[:],
                            lhsT=wbT(k, g),
                            rhs=seT(k),
                            start=(k == 0),
                            stop=(k == NK - 1),
                        )
                    for t in range(T):
                        dst = Bsc[g][:, t * NB : (t + 1) * NB]
                        srcp = bs_psum[:, t * NB : (t + 1) * NB]
                        if t == 0:
                            nc.vector.tensor_scalar(dst, srcp, 0.5, None, OP.mult)
                        else:
                            nc.vector.scalar_tensor_tensor(
                                dst, srcp, float(2 ** (t - 1)),
                                Bsc[g][:, (t - 1) * NB : t * NB],
                                OP.mult, OP.add,
                            )

            def emit_sub(pair, sub):
                t = 2 * pair + sub
                sc_t = float(2 ** t)
                q_b16 = []
                for g in range(NG):
                    At = A[g][t % 2]
                    if t == 0:
                        nc.vector.tensor_copy(M[g][:], At[:])
                    else:
                        nc.vector.tensor_tensor(M[g][:], At[:], M[g][:], OP.add)
                    b_bc = (
                        Bsc[g][:, t * NB : (t + 1) * NB]
                        .unsqueeze(2)
                        .broadcast_to([128, NB, S])
                    )
                    m_v = M[g].rearrange("p (b s) -> p b s", s=S)
                    nc.vector.tensor_tensor(m_v, b_bc, m_v, OP.add)
                    qg = qpool.tile([128, R], BF16, tag=f"q{g}", name="qg")
                    q_b16.append(qg)
                    nc.vector.tensor_scalar(
                        qg[:], M[g][:], float(2 ** (t + 1)), None, OP.is_le
                    )
                    nc.vector.tensor_tensor(M[g][:], M[g][:], qg[:], OP.mult)

                hq_psum = []
                for g in range(NG):
                    ps = hpool.tile([128, R], F32, tag="hq", name="hq_psum")
                    hq_psum.append(ps)
                    for k in range(NG):
                        nc.tensor.matmul(
                            ps[:],
                            lhsT=w1T(k, g),
                            rhs=q_b16[k][:],
                            start=(k == 0),
                            stop=(k == NG - 1),
                        )

                sp2_b16 = []
                for g in range(NG):
                    if t == 0:
                        nc.vector.tensor_scalar(
                            ML[g][:], hq_psum[g][:], -1.0, None, OP.mult
                        )
                    else:
                        nc.vector.scalar_tensor_tensor(
                            ML[g][:], hq_psum[g][:], -sc_t, ML[g][:], OP.mult, OP.add
                        )
                    nc.scalar.activation(
                        ML[g][:], ML[g][:],
                        mybir.ActivationFunctionType.Identity,
                        bias=c1s_ap(g, t), scale=1.0,
                    )
                    spg = qpool.tile([128, R], BF16, tag=f"sp2{g}", name="spg")
                    sp2_b16.append(spg)
                    nc.vector.tensor_scalar(spg[:], ML[g][:], sc_t, None, OP.is_gt)
                    nc.vector.scalar_tensor_tensor(
                        ML[g][:], ML[g][:], sc_t, ML[g][:], OP.is_le, OP.mult
                    )

                for k in range(NG):
                    nc.tensor.matmul(
                        o_psum[:],
                        lhsT=w2T(k),
                        rhs=sp2_b16[k][:],
                        start=(t == 0 and k == 0),
                        stop=(t == T - 1 and k == NG - 1),
                    )

            # prologue: pair 0 load + apical
            te_tiles, last_te_dma = emit_te_dma(0, chain=True)
            cur_psum = [
                appool.tile([128, 2 * R], F32, tag="ap", name="ap_psum")
                for _ in range(NG)
            ]
            for c in range(NCHUNK):
                emit_ap_chunk(cur_psum, te_tiles, c)
            dM = nc.sync.dma_start(wallM_sb[:], wallM[:])
            add_dep_helper(dM.ins, last_te_dma.ins, reason="wallM after te0 chain")
            dB = nc.sync.dma_start(wallB_sb[:], wallB[:])
            add_dep_helper(dB.ins, dM.ins, reason="wallB after wallM")
            dC = nc.sync.dma_start(cons_sb[:], cons[:])
            add_dep_helper(dC.ins, dM.ins, reason="cons after wallM")
            emit_basal()

            for pair in range(NPAIR):
                emit_a_updates(cur_psum, pair)
                if pair + 1 < NPAIR:
                    te_tiles, _ = emit_te_dma(pair + 1, chain=False)
                    nxt_psum = [
                        appool.tile([128, 2 * R], F32, tag="ap", name="ap_psum")
                        for _ in range(NG)
                    ]
                    # interleave next-pair apical chunks with this pair's
                    # recurrent chain
                    emit_ap_chunk(nxt_psum, te_tiles, 0)
                    emit_ap_chunk(nxt_psum, te_tiles, 1)
                    emit_sub(pair, 0)
                    emit_ap_chunk(nxt_psum, te_tiles, 2)
                    emit_ap_chunk(nxt_psum, te_tiles, 3)
                    emit_sub(pair, 1)
                    emit_ap_chunk(nxt_psum, te_tiles, 4)
                    emit_ap_chunk(nxt_psum, te_tiles, 5)
                    cur_psum = nxt_psum
                else:
                    emit_sub(pair, 0)
                    emit_sub(pair, 1)

            # ---- final eviction: out = o_psum / T + b2 ----
            out_sb = state.tile([L, R], F32, tag="out_sb", name="out_sb")
            nc.scalar.activation(
                out_sb[:], o_psum[:],
                mybir.ActivationFunctionType.Identity,
                bias=b2_ap, scale=1.0 / T,
            )
            nc.sync.dma_start(out[:], out_sb[:])

    return nc


def _swizzle_kmaj(a, cols):
    """[KD-like rows, cols] fp -> [128, nk*cols] bf16 with [p, k*cols+c]=a[k*128+p, c]"""
    bf = ml_dtypes.bfloat16
    nk = a.shape[0] // 128
    return np.ascontiguousarray(
        a.reshape(nk, 128, cols).transpose(1, 0, 2).reshape(128, nk * cols).astype(bf)
    )


def prep_in_maps(inputs):
    """Host-side shard + transpose + pad + cast. Returns list of per-core dicts."""
    se = np.asarray(inputs["state_embedding"], np.float32)
    te = np.asarray(inputs["tau_embedding"], np.float32)
    Wb = np.asarray(inputs["Wb"], np.float32)
    Wa = np.asarray(inputs["Wa"], np.float32)
    W1 = np.asarray(inputs["W1"], np.float32)
    b1 = np.asarray(inputs["b1"], np.float32)
    W2 = np.asarray(inputs["W2"], np.float32)
    b2 = np.asarray(inputs["b2"], np.float32)
    bf = ml_dtypes.bfloat16

    def padk(a):  # pad feature axis 0 from 3136 to KD
        o = np.zeros((KD,) + a.shape[1:], a.dtype)
        o[: a.shape[0]] = a
        return o

    wallA = _swizzle_kmaj(padk(Wa.T), F)
    wallM_wb = _swizzle_kmaj(padk(Wb.T), F)
    wallB = np.empty((128, WB_COLS), bf)
    wallB[:, O_W1 : O_W1 + NG * H] = _swizzle_kmaj(np.ascontiguousarray(W1.T), H)
    wallB[:, O_W2 : O_W2 + NG * L] = _swizzle_kmaj(np.ascontiguousarray(W2.T), L)

    cons = np.zeros((128, NG * T + 1 + 2 * T), np.float32)
    c1 = W1.sum(axis=1) + b1
    for g in range(NG):
        for t in range(T):
            cons[:, g * T + t] = c1[g * 128 : (g + 1) * 128] * (2.0 ** t)
    cons[:L, NG * T] = b2
    for t in range(T):
        cons[:, NG * T + 1 + t] = -(2.0 ** (t + 1))
        cons[:, NG * T + 1 + T + t] = -(2.0 ** t)

    in_maps = []
    for i in range(N_CORES):
        # teT: [NPAIR, 128, NK*2R] with [pair, p, k*512 + (sub*R+r)] = te[t, row, d]
        tei = te[:, i * R : (i + 1) * R, :]       # [T, R, DT]
        tei = tei.reshape(NPAIR, 2 * R, DT)       # [pair, sub*R+r, d]
        tei_p = np.zeros((NPAIR, 2 * R, KD), np.float32)
        tei_p[:, :, :DT] = tei
        teT = np.ascontiguousarray(
            tei_p.reshape(NPAIR, 2 * R, NK, 128)
            .transpose(0, 3, 2, 1)                # [pair, p, k, n]
            .reshape(NPAIR, 128, NK * 2 * R)
            .astype(bf)
        )
        # seT region of wall: [p, k*T*NB + t*NB+b] = se[t, batch, d]
        sei = se[:, i * NB : (i + 1) * NB, :]     # [T, NB, DS]
        seT = padk(np.ascontiguousarray(sei.reshape(T * NB, DS).T))  # [KD, T*NB]
        wallM_i = np.empty((128, WM_COLS), bf)
        wallM_i[:, O_WB : O_WB + NK * F] = wallM_wb
        wallM_i[:, O_SE : O_SE + NK * T * NB] = _swizzle_kmaj(seT, T * NB)
        in_maps.append(dict(teT=teT, wallA=wallA, wallM=wallM_i, wallB=wallB, cons=cons))
    return in_maps


def assemble_out(core_outs):
    """[N_CORES][L, R] -> [B, L, S]"""
    full = np.stack([np.asarray(o, np.float32) for o in core_outs], axis=0)
    full = full.reshape(N_CORES, L, NB, S).transpose(0, 2, 1, 3)
    return np.ascontiguousarray(full.reshape(B, L, S))


_NC_CACHE = {}


def get_nc(with_b1=False, state_dt=BF16):
    key = ("nc", str(state_dt))
    if key not in _NC_CACHE:
        last = None
        for _ in range(6):
            try:
                _NC_CACHE[key] = build_nc(state_dt=state_dt)
                break
            except Exception as e:  # rare scheduler-order race-detector trip
                last = e
        else:
            raise last
    return _NC_CACHE[key]


def run_sharded(in_maps, with_b1=False, trace=False, **kw):
    nc = get_nc(with_b1=with_b1)
    if not getattr(nc, "_waits_split", False):
        _split_excess_waits(nc)
        nc._waits_split = True
    res = run_bass_kernel_spmd(
        nc, in_maps, core_ids=list(range(N_CORES)), trace=trace, **kw
    )
    return res


def kernel(**inputs):
    in_maps = prep_in_maps(inputs)
    with_b1 = bool(np.any(np.asarray(inputs["b1"], np.float32)))
    res = run_sharded(in_maps, with_b1=with_b1)
    return assemble_out([res.results[i]["out"] for i in range(N_CORES)])



# revision 2
# speedup vs baseline: 1.0107x; 1.0107x over previous
"""Trainium2 Bass kernel for nn_MCQuantiles (ThreeCompNode SNN scan) — v3.

Strategy:
- All heavy matmuls in fp8 e4m3 with DoubleRow perf mode (2 k-tiles per
  instruction = 2x effective PE throughput vs bf16). K padded 3136->3328
  (26 k-tiles = 13 DoubleRow pairs).
- The apical membrane's cumulative sum is folded into the matmul: host
  sends, for substep t, the column 16*sum_{tau<=t} 2^(tau-t)*te_tau, so
  PSUM holds 1024*2^-(t-1)*A_t directly. The basal cumulative term (same
  host presum over se) is injected into the same PSUM banks by a [128x128]
  identity matmul with an S-broadcast rhs. Layer-1 per step is then just
  one stt from PSUM (M = psum*2^(t-1)/1024 + M'), one is_gt -> fp8 spike,
  one is_le*mult reset, all merged [128, 1024] across the 4 f-groups.
- LIF layer: stt from hq PSUM, is_gt -> fp8 sp2 (stored per t), is_le*mult
  reset. W2 runs as a 16-instruction tail over the stored sp2 tiles.
- PSUM: 2 rotating [128, NG, 2R] fp32 tiles (4 banks each = all 8 banks).
  hq and W2 outputs are carved into banks of the pair tile whose apical
  data is already consumed (the two M-updates of a pair run before any hq
  matmul so start=True bank zeroing never kills live data).
"""
import numpy as np
import ml_dtypes

import bass_rust
import concourse.bass as bass
import concourse.mybir as mybir
from concourse.bass_utils import run_bass_kernel_spmd
from concourse.tile import TileContext
from concourse.tile_rust import add_dep_helper

# ----- problem constants (hardcoded per contract) -----
T, B, S = 8, 64, 32
DS = DT = 3136
F = H = 512
L = 18
N_CORES = 8
NB = B // N_CORES              # 8 batches per core
R = NB * S                     # 256 rows per core
NK = 26                        # k-tiles of 128 (3136 padded to 3328)
KD = NK * 128
NKP = NK // 2                  # 13 DoubleRow k-pairs
NPAIR = T // 2                 # 4 time-step pairs
NG = F // 128                  # 4 feature groups
TNB = T * NB                   # 64 basal columns
LP = 32                        # L padded for DoubleRow lhsT stride alignment

CH_K = [2, 4, 4, 4, 6, 6]      # k-tiles per DMA chunk (26 total, pair-aligned)
CH_OFF = [0, 2, 6, 10, 14, 20]
NCHUNK = len(CH_K)

F32 = mybir.dt.float32
BF16 = mybir.dt.bfloat16
FP8 = mybir.dt.float8e4
OP = mybir.AluOpType
DR = mybir.MatmulPerfMode.DoubleRow
AF = mybir.ActivationFunctionType

SC_IN = 16.0                   # activation host scale
SC_W = 64.0                    # weight host scale
SC_AP = SC_IN * SC_W           # apical/basal psum carry 1024x


def _patch_tile_drain():
    """This walrus build allows a single sync-wait per TPB_CTRL Drain; Tile's
    kernel-tail drain attaches one wait per active logical proc. Split them
    across a chain of drains."""
    def _patched(self, tick_clock, wait_clock):
        nc = self.nc
        drain_inst = nc.sync.drain()
        wait_clock.add_sem_waits(
            drain_inst.ins, bass_rust.ScopedClock({None: tick_clock.global_clock})
        )
        si = drain_inst.ins.sync_info
        if si is not None and len(si.on_wait) > 1:
            waits = list(si.on_wait)
            drain_inst.ins.sync_info = mybir.SyncInfo(
                on_wait=waits[:1], on_update=list(si.on_update)
            )
            for w in waits[1:]:
                extra = nc.sync.drain()
                extra.ins.sync_info = mybir.SyncInfo(on_wait=[w], on_update=[])
        nc.all_engine_barrier()
        popped = nc._tile_sem_poison_stack.pop()
        assert popped is self._sem_poison
        nc.clear_and_free_semaphores(list(self.sems.allocated().values()))
        nc.all_engine_barrier()

    TileContext._drain_and_barrier = _patched


def _split_excess_waits(nc, limit=1):
    """Walrus here rejects instructions carrying more than ~1 sync-wait. Move
    excess waits onto same-engine NoOps inserted just before the instruction."""
    for fn in nc.m.functions:
        for bb in fn.blocks:
            new = []
            changed = False
            for inst in bb.instructions:
                si = getattr(inst, "sync_info", None)
                ow = list(si.on_wait) if si is not None and si.on_wait else []
                if len(ow) > limit:
                    extra = ow[limit:]
                    for j in range(0, len(extra), limit):
                        nop = mybir.InstNoOp(
                            name=f"{inst.name}-ws{j}", ins=[], outs=[]
                        )
                        nop.engine = inst.engine
                        nop.sync_info = mybir.SyncInfo(
                            on_wait=extra[j : j + limit], on_update=[]
                        )
                        new.append(nop)
                    inst.sync_info = mybir.SyncInfo(
                        on_wait=ow[:limit], on_update=list(si.on_update)
                    )
                    changed = True
                new.append(inst)
            if changed:
                try:
                    bb.instructions[:] = new
                except TypeError:
                    bb.instructions = new


def build_nc():
    _patch_tile_drain()
    nc = bass.Bass()

    teT = nc.declare_dram_parameter("teT", [NPAIR, 128, NK * 2 * R], FP8, isOutput=False)
    wallA = nc.declare_dram_parameter("wallA", [128, NK * F], FP8, isOutput=False)
    wallM = nc.declare_dram_parameter("wallM", [128, NK * F + NK * TNB], FP8, isOutput=False)
    wallB = nc.declare_dram_parameter("wallB", [128, NG * H + NG * LP], FP8, isOutput=False)
    cons = nc.declare_dram_parameter("cons", [128, 2 + 3 * T], F32, isOutput=False)
    ident = nc.declare_dram_parameter("ident", [128, 128], BF16, isOutput=False)
    out = nc.declare_dram_parameter("out", [L, R], F32, isOutput=True)

    with TileContext(nc) as tc:
        with (
            tc.tile_pool(name="wpool", bufs=1) as wpool,
            tc.tile_pool(name="tepool", bufs=2) as tepool,
            tc.tile_pool(name="state", bufs=1) as state,
            tc.tile_pool(name="sppool", bufs=2) as sppool,
            tc.tile_pool(name="rpool", bufs=2) as rpool,
            tc.tile_pool(name="appool", bufs=2, space="PSUM") as appool,
        ):
            # ---- resident weights ----
            WA_CH = [2, 4, 8, 12]  # k-tiles per wallA chunk
            WA_OFF = [0, 2, 6, 14]
            wa_ck = [
                wpool.tile([128, WA_CH[c], F], FP8, tag=f"wallA{c}", name=f"wa{c}")
                for c in range(len(WA_CH))
            ]

            wallWb = wpool.tile([128, NK, F], FP8, tag="wallWb", name="wallWb")
            wallSe = wpool.tile([128, NK, TNB], FP8, tag="wallSe", name="wallSe")
            wallW1 = wpool.tile([128, NG, H], FP8, tag="wallW1", name="wallW1")
            wallW2 = wpool.tile([128, NG, LP], FP8, tag="wallW2", name="wallW2")
            cons_sb = wpool.tile([128, 2 + 3 * T], F32, tag="cons", name="cons_sb")
            ident_sb = wpool.tile([128, 128], BF16, tag="ident", name="ident_sb")

            def waT(jp, g):
                k = 2 * jp
                for c in range(len(WA_CH) - 1, -1, -1):
                    if k >= WA_OFF[c]:
                        kk = k - WA_OFF[c]
                        return wa_ck[c][:, kk : kk + 2, g * 128 : (g + 1) * 128]

            # ---- states ----
            M = state.tile([128, NG * R], BF16, tag="M", name="M")
            ML = state.tile([128, NG * R], BF16, tag="ML", name="ML")
            bs_sb = state.tile([128, NG, TNB], BF16, tag="bs", name="bs_sb")
            sp2s = [state.tile([128, NG, R], FP8, tag=f"sp2_{t}", name=f"sp2_{t}")
                    for t in range(T)]

            nc.vector.memset(M[:], 0.0)
            nc.gpsimd.memset(ML[:], 0.0)

            # ---- te DMA ----
            TE0_CH = [2, 4, 8, 12]   # k-tiles per pair-0 chunk
            TE0_OFF = [0, 2, 6, 14]

            def emit_ap0_chunk(psum, te_tiles, c):
                for kk in range(TE0_CH[c] // 2):
                    jp = TE0_OFF[c] // 2 + kk
                    for g in range(NG):
                        nc.tensor.matmul(
                            psum[:, g, :],
                            lhsT=waT(jp, g),
                            rhs=te_tiles[c][:, 2 * kk : 2 * kk + 2, :],
                            start=(jp == 0),
                            stop=False,
                            perf_mode=DR,
                        )

            def emit_ap_group(psum, te_tile, jp_lo, jp_hi):
                for jp in range(jp_lo, jp_hi):
                    for g in range(NG):
                        nc.tensor.matmul(
                            psum[:, g, :],
                            lhsT=waT(jp, g),
                            rhs=te_tile[:, 2 * jp : 2 * jp + 2, :],
                            start=(jp == 0),
                            stop=False,
                            perf_mode=DR,
                        )

            def emit_ident(psum, pair):
                # inject scaled cumulative basal, broadcast over S; closes the
                # psum accumulation group of each bank (stop=True)
                for g in range(NG):
                    rhs = (
                        bs_sb[:, g, :]
                        .rearrange("p (t b) -> p t b", b=NB)[:, 2 * pair : 2 * pair + 2, :]
                        .unsqueeze(3)
                        .broadcast_to([128, 2, NB, S])
                    )
                    nc.tensor.matmul(
                        psum[:, g, :],
                        lhsT=ident_sb[:],
                        rhs=rhs,
                        start=False,
                        stop=True,
                    )

            # ---- per-substep emitters ----
            def emit_sub(psum, t):
                sub = t % 2
                th1 = float(2 ** (t + 1))
                src = psum[:, :, sub * R : (sub + 1) * R]
                # M = psum*2^(t-1)/1024 + M'  (psum holds scaled A_t + B_t)
                mstt = nc.vector.scalar_tensor_tensor(
                    M[:], src, float(2 ** (t - 1)) / SC_AP, M[:],
                    OP.mult, OP.add,
                )
                sp = sppool.tile([128, NG, R], FP8, tag="sp", name="sp")
                nc.vector.tensor_scalar(sp[:], M[:], th1, None, OP.is_gt)
                nc.vector.scalar_tensor_tensor(
                    M[:], M[:], th1, M[:], OP.is_le, OP.mult
                )
                return sp, mstt

            def emit_w1(psum, sp, t):
                sub = t % 2
                first = None
                for g in range(NG):
                    bank = 2 * sub + g // 2
                    dst = psum[:, bank, (g % 2) * R : (g % 2 + 1) * R]
                    for j in range(2):
                        mm = nc.tensor.matmul(
                            dst,
                            lhsT=wallW1[:, 2 * j : 2 * j + 2, g * 128 : (g + 1) * 128],
                            rhs=sp[:, 2 * j : 2 * j + 2, :],
                            start=(g % 2 == 0 and j == 0),
                            stop=(g % 2 == 1 and j == 1),
                            perf_mode=DR,
                        )
                        if first is None:
                            first = mm
                return first

            def emit_l2(psum, t):
                sub = t % 2
                th2 = float(2 ** t)
                hq = psum[:, 2 * sub : 2 * sub + 2, :].rearrange("p a b -> p (a b)")
                nc.vector.scalar_tensor_tensor(
                    ML[:], hq, float(2 ** t) / SC_W, ML[:], OP.mult, OP.add
                )
                nc.vector.tensor_scalar(sp2s[t][:], ML[:], th2, None, OP.is_gt)
                nc.vector.scalar_tensor_tensor(
                    ML[:], ML[:], th2, ML[:], OP.is_le, OP.mult
                )

            # ================= schedule =================
            # One serialized DMA chain delivering bytes in PE consumption
            # order: te0/wallA chunks interleaved, basal weights early.
            te0_tiles = [
                tepool.tile([128, TE0_CH[c], 2 * R], FP8, tag=f"te0_{c}",
                            name=f"te0_{c}")
                for c in range(len(TE0_CH))
            ]

            def te0_dma(c):
                return nc.sync.dma_start(
                    te0_tiles[c][:],
                    teT[0][:, TE0_OFF[c] * 2 * R
                           : (TE0_OFF[c] + TE0_CH[c]) * 2 * R],
                )

            def wa_dma(c):
                return nc.sync.dma_start(
                    wa_ck[c][:],
                    wallA[:, WA_OFF[c] * F : (WA_OFF[c] + WA_CH[c]) * F],
                )

            # Sync executes triggers in emission order, so emit them in PE
            # need-order; per-chain deps keep each stream serialized.
            chain_te, chain_wa, chain_wb = [], [], []
            chain_te.append(te0_dma(0))
            chain_wa.append(wa_dma(0))
            chain_te.append(te0_dma(1))
            chain_wa.append(wa_dma(1))
            chain_wb.append(nc.sync.dma_start(wallWb[:], wallM[:, : NK * F]))
            chain_wb.append(nc.sync.dma_start(wallSe[:], wallM[:, NK * F :]))
            chain_te.append(te0_dma(2))
            chain_wa.append(wa_dma(2))
            chain_te.append(te0_dma(3))
            chain_wa.append(wa_dma(3))
            chain_wb.append(nc.sync.dma_start(wallW1[:], wallB[:, : NG * H]))
            chain_wb.append(nc.sync.dma_start(wallW2[:], wallB[:, NG * H :]))
            chain_wb.append(nc.sync.dma_start(cons_sb[:], cons[:]))
            chain_wb.append(nc.sync.dma_start(ident_sb[:], ident[:]))
            for a, b in zip(chain_wb, chain_wb[1:]):
                add_dep_helper(b.ins, a.ins, reason="prologue dma chain")

            ap_cur = appool.tile([128, NG, 2 * R], F32, tag="ap", name="ap0")
            ap_nxt = appool.tile([128, NG, 2 * R], F32, tag="ap", name="ap1")

            def emit_basal(j_lo, j_hi):
                # basal matmuls into ap_nxt banks (consumed+copied before
                # pair1 apical start=True re-zeroes them); columns hold the
                # scaled cumulative basal per substep (host presums se)
                for g in range(NG):
                    for j in range(j_lo, j_hi):
                        nc.tensor.matmul(
                            ap_nxt[:, g, :TNB],
                            lhsT=wallWb[:, 2 * j : 2 * j + 2,
                                        g * 128 : (g + 1) * 128],
                            rhs=wallSe[:, 2 * j : 2 * j + 2, :],
                            start=(j == 0),
                            stop=(j == NKP - 1),
                            perf_mode=DR,
                        )

            emit_ap0_chunk(ap_cur, te0_tiles, 0)
            emit_ap0_chunk(ap_cur, te0_tiles, 1)
            emit_ap0_chunk(ap_cur, te0_tiles, 2)
            emit_basal(0, NKP)
            emit_ap0_chunk(ap_cur, te0_tiles, 3)
            nc.vector.tensor_copy(bs_sb[:], ap_nxt[:, :, :TNB])
            emit_ident(ap_cur, 0)


            # main loop
            def emit_w2(t):
                # o accumulates in a dead bank of the pair-2 psum tile
                for j in range(2):
                    nc.tensor.matmul(
                        o_ps,
                        lhsT=wallW2[:, 2 * j : 2 * j + 2, :],
                        rhs=sp2s[t][:, 2 * j : 2 * j + 2, :],
                        start=(t == 0 and j == 0),
                        stop=(t == T - 1 and j == 1),
                        perf_mode=DR,
                    )

            teP = None
            o_ps = None
            for pair in range(NPAIR):
                t0, t1 = 2 * pair, 2 * pair + 1
                last = pair + 1 == NPAIR
                if not last:
                    teP = tepool.tile([128, NK, 2 * R], FP8, tag="teP",
                                      name=f"teP{pair + 1}")
                    for lo, hi in ((0, 10), (10, 18), (18, NK)):
                        nc.sync.dma_start(
                            teP[:, lo:hi, :],
                            teT[pair + 1][:, lo * 2 * R : hi * 2 * R],
                        )
                sp0, mstt0 = emit_sub(ap_cur, t0)
                sp1, mstt1 = emit_sub(ap_cur, t1)
                if not last:
                    emit_ap_group(ap_nxt, teP, 0, 5)
                else:
                    for t in range(0, 3):
                        emit_w2(t)
                w1_first = emit_w1(ap_cur, sp0, t0)
                # W1(t0)'s start=True zeroes whole banks incl. the halves
                # Mstt(t1) reads; no AP overlap so force the order explicitly
                add_dep_helper(w1_first.ins, mstt1.ins,
                               reason="hq bank zero after Mstt t1")
                emit_l2(ap_cur, t0)
                if not last:
                    emit_ap_group(ap_nxt, teP, 5, 9)
                else:
                    for t in range(3, 6):
                        emit_w2(t)
                emit_w1(ap_cur, sp1, t1)
                emit_l2(ap_cur, t1)
                if not last:
                    emit_ap_group(ap_nxt, teP, 9, NKP)
                    emit_ident(ap_nxt, pair + 1)
                if pair + 2 < NPAIR:
                    ap_cur, ap_nxt = ap_nxt, appool.tile(
                        [128, NG, 2 * R], F32, tag="ap", name=f"ap{pair + 2}"
                    )
                elif pair + 2 == NPAIR:
                    # ap_cur (pair-2 tile) retires after this pair; bank 0
                    # hosts the W2 output accumulation
                    o_ps = ap_cur[0:LP, 0, 0:R]
                    ap_cur, ap_nxt = ap_nxt, ap_cur

            emit_w2(6)
            emit_w2(7)
            out_sb = state.tile([L, R], F32, tag="out_sb", name="out_sb")
            nc.scalar.activation(
                out_sb[:], ap_nxt[0:L, 0, 0:R], AF.Identity,
                bias=cons_sb[0:L, 0:1], scale=1.0 / (T * SC_W),
            )
            nc.sync.dma_start(out[:], out_sb[:])

    return nc


# ---------------- host side ----------------

def _padk(a, kd=KD):
    o = np.zeros((kd,) + a.shape[1:], a.dtype)
    o[: a.shape[0]] = a
    return o


def _kmaj(a, cols):
    """[KD, cols] f32 -> [128, NK*cols] fp8, [p, k*cols+c] = a[k*128+p, c]"""
    e4 = ml_dtypes.float8_e4m3
    nk = a.shape[0] // 128
    return np.ascontiguousarray(
        a.reshape(nk, 128, cols).transpose(1, 0, 2).reshape(128, nk * cols)
    ).astype(e4)


def prep_in_maps(inputs):
    se = np.asarray(inputs["state_embedding"], np.float32)
    te = np.asarray(inputs["tau_embedding"], np.float32)
    Wb = np.asarray(inputs["Wb"], np.float32)
    Wa = np.asarray(inputs["Wa"], np.float32)
    W1 = np.asarray(inputs["W1"], np.float32)
    b1 = np.asarray(inputs["b1"], np.float32)
    W2 = np.asarray(inputs["W2"], np.float32)
    b2 = np.asarray(inputs["b2"], np.float32)
    e4 = ml_dtypes.float8_e4m3

    wallA_np = _kmaj(_padk(Wa.T * SC_W), F)
    wallWb_np = _kmaj(_padk(Wb.T * SC_W), F)
    wallM_np = np.empty((128, NK * F + NK * TNB), e4)
    wallM_np[:, : NK * F] = wallWb_np

    wallB_np = np.empty((128, NG * H + NG * LP), e4)
    wallB_np[:, : NG * H] = _kmaj(np.ascontiguousarray(W1.T) * SC_W, H)
    w2p = np.zeros((H, LP), np.float32)
    w2p[:, :L] = W2.T * SC_W
    wallB_np[:, NG * H :] = _kmaj(w2p, LP)

    # fold the A/B cumulative recurrences into the matmul columns:
    # column for substep t carries 16 * sum_{tau<=t} 2^(tau-t) * x_tau
    wts = (2.0 ** np.arange(T)).astype(np.float32)[:, None, None]
    te = np.cumsum(te * wts, axis=0) / wts
    se = np.cumsum(se * wts, axis=0) / wts

    ident_np = np.eye(128, dtype=ml_dtypes.bfloat16)

    cons_np = np.zeros((128, 2 + 3 * T), np.float32)
    cons_np[:L, 0] = b2
    for t in range(T):
        cons_np[:, 2 + t] = -(2.0 ** t)          # relu bias
        cons_np[:, 2 + T + t] = 2.0 ** (t + 1)   # layer-1 spike threshold
        cons_np[:, 2 + 2 * T + t] = 2.0 ** t     # LIF reset threshold
    # b1 is zero in this problem; assert so silent wrongness can't slip in
    assert not np.any(b1), "kernel assumes b1 == 0"

    in_maps = []
    for i in range(N_CORES):
        tei = te[:, i * R : (i + 1) * R, :]                  # [T, R, DT]
        tei = (tei * SC_IN).reshape(NPAIR, 2, R, DT)
        tei_p = np.zeros((NPAIR, 2, R, KD), np.float32)
        tei_p[..., :DT] = tei
        teT_np = np.ascontiguousarray(
            tei_p.reshape(NPAIR, 2, R, NK, 128)
            .transpose(0, 4, 3, 1, 2)                        # [pair,p,k,sub,r]
            .reshape(NPAIR, 128, NK * 2 * R)
        ).astype(e4)

        sei = se[:, i * NB : (i + 1) * NB, :]                # [T, NB, DS]
        seT = _padk(np.ascontiguousarray(sei.reshape(TNB, DS).T) * SC_IN)
        wallM_i = wallM_np.copy()
        wallM_i[:, NK * F :] = _kmaj(seT, TNB)
        in_maps.append(dict(teT=teT_np, wallA=wallA_np, wallM=wallM_i,
                            wallB=wallB_np, cons=cons_np, ident=ident_np))
    return in_maps


def assemble_out(core_outs):
    full = np.stack([np.asarray(o, np.float32) for o in core_outs], axis=0)
    full = full.reshape(N_CORES, L, NB, S).transpose(0, 2, 1, 3)
    return np.ascontiguousarray(full.reshape(B, L, S))


_NC_CACHE = {}


def get_nc():
    if "nc" not in _NC_CACHE:
        last = None
        for _ in range(6):
            try:
                _NC_CACHE["nc"] = build_nc()
                break
            except Exception as e:  # rare scheduler-order race-detector trip
                last = e
        else:
            raise last
    return _NC_CACHE["nc"]


def run_sharded(in_maps, trace=False, **kw):
    nc = get_nc()
    if not getattr(nc, "_waits_split", False):
        _split_excess_waits(nc)
        nc._waits_split = True
    return run_bass_kernel_spmd(
        nc, in_maps, core_ids=list(range(N_CORES)), trace=trace, **kw
    )


def kernel(**inputs):
    in_maps = prep_in_maps(inputs)
    res = run_sharded(in_maps)
    return assemble_out([res.results[i]["out"] for i in range(N_CORES)])


# revision 3
# speedup vs baseline: 1.0210x; 1.0101x over previous
"""Trainium2 Bass kernel for nn_MCQuantiles (ThreeCompNode SNN scan) — v3.

Strategy:
- All heavy matmuls in fp8 e4m3 with DoubleRow perf mode (2 k-tiles per
  instruction = 2x effective PE throughput vs bf16). K padded 3136->3328
  (26 k-tiles = 13 DoubleRow pairs).
- The apical membrane's cumulative sum is folded into the matmul: host
  sends, for substep t, the column 16*sum_{tau<=t} 2^(tau-t)*te_tau, so
  PSUM holds 1024*2^-(t-1)*A_t directly. The basal cumulative term (same
  host presum over se) is injected into the same PSUM banks by a [128x128]
  identity matmul with an S-broadcast rhs. Layer-1 per step is then just
  one stt from PSUM (M = psum*2^(t-1)/1024 + M'), one is_gt -> fp8 spike,
  one is_le*mult reset, all merged [128, 1024] across the 4 f-groups.
- LIF layer: stt from hq PSUM, is_gt -> fp8 sp2 (stored per t), is_le*mult
  reset. W2 runs as a 16-instruction tail over the stored sp2 tiles.
- PSUM: 2 rotating [128, NG, 2R] fp32 tiles (4 banks each = all 8 banks).
  hq and W2 outputs are carved into banks of the pair tile whose apical
  data is already consumed (the two M-updates of a pair run before any hq
  matmul so start=True bank zeroing never kills live data).
"""
import numpy as np
import ml_dtypes

import bass_rust
import concourse.bass as bass
import concourse.mybir as mybir
from concourse.bass_utils import run_bass_kernel_spmd
from concourse.tile import TileContext
from concourse.tile_rust import add_dep_helper

# ----- problem constants (hardcoded per contract) -----
T, B, S = 8, 64, 32
DS = DT = 3136
F = H = 512
L = 18
N_CORES = 8
NB = B // N_CORES              # 8 batches per core
R = NB * S                     # 256 rows per core
NK = 26                        # k-tiles of 128 (3136 padded to 3328)
KD = NK * 128
NKP = NK // 2                  # 13 DoubleRow k-pairs
NPAIR = T // 2                 # 4 time-step pairs
NG = F // 128                  # 4 feature groups
TNB = T * NB                   # 64 basal columns
LP = 32                        # L padded for DoubleRow lhsT stride alignment

CH_K = [2, 4, 4, 4, 6, 6]      # k-tiles per DMA chunk (26 total, pair-aligned)
CH_OFF = [0, 2, 6, 10, 14, 20]
NCHUNK = len(CH_K)

F32 = mybir.dt.float32
BF16 = mybir.dt.bfloat16
FP8 = mybir.dt.float8e4
OP = mybir.AluOpType
DR = mybir.MatmulPerfMode.DoubleRow
AF = mybir.ActivationFunctionType

SC_IN = 16.0                   # activation host scale
SC_W = 64.0                    # weight host scale
SC_AP = SC_IN * SC_W           # apical/basal psum carry 1024x


def _patch_tile_drain():
    """This walrus build allows a single sync-wait per TPB_CTRL Drain; Tile's
    kernel-tail drain attaches one wait per active logical proc. Split them
    across a chain of drains."""
    def _patched(self, tick_clock, wait_clock):
        nc = self.nc
        drain_inst = nc.sync.drain()
        wait_clock.add_sem_waits(
            drain_inst.ins, bass_rust.ScopedClock({None: tick_clock.global_clock})
        )
        si = drain_inst.ins.sync_info
        if si is not None and len(si.on_wait) > 1:
            waits = list(si.on_wait)
            drain_inst.ins.sync_info = mybir.SyncInfo(
                on_wait=waits[:1], on_update=list(si.on_update)
            )
            for w in waits[1:]:
                extra = nc.sync.drain()
                extra.ins.sync_info = mybir.SyncInfo(on_wait=[w], on_update=[])
        nc.all_engine_barrier()
        popped = nc._tile_sem_poison_stack.pop()
        assert popped is self._sem_poison
        nc.clear_and_free_semaphores(list(self.sems.allocated().values()))
        nc.all_engine_barrier()

    TileContext._drain_and_barrier = _patched


def _split_excess_waits(nc, limit=1):
    """Walrus here rejects instructions carrying more than ~1 sync-wait. Move
    excess waits onto same-engine NoOps inserted just before the instruction."""
    for fn in nc.m.functions:
        for bb in fn.blocks:
            new = []
            changed = False
            for inst in bb.instructions:
                si = getattr(inst, "sync_info", None)
                ow = list(si.on_wait) if si is not None and si.on_wait else []
                if len(ow) > limit:
                    extra = ow[limit:]
                    for j in range(0, len(extra), limit):
                        nop = mybir.InstNoOp(
                            name=f"{inst.name}-ws{j}", ins=[], outs=[]
                        )
                        nop.engine = inst.engine
                        nop.sync_info = mybir.SyncInfo(
                            on_wait=extra[j : j + limit], on_update=[]
                        )
                        new.append(nop)
                    inst.sync_info = mybir.SyncInfo(
                        on_wait=ow[:limit], on_update=list(si.on_update)
                    )
                    changed = True
                new.append(inst)
            if changed:
                try:
                    bb.instructions[:] = new
                except TypeError:
                    bb.instructions = new


def build_nc():
    _patch_tile_drain()
    nc = bass.Bass()

    teT = nc.declare_dram_parameter("teT", [NPAIR, 128, NK * 2 * R], FP8, isOutput=False)
    wallA = nc.declare_dram_parameter("wallA", [128, NK * F], FP8, isOutput=False)
    wallM = nc.declare_dram_parameter("wallM", [128, NK * F + NK * TNB], FP8, isOutput=False)
    wallB = nc.declare_dram_parameter("wallB", [128, NG * H + NG * LP], FP8, isOutput=False)
    cons = nc.declare_dram_parameter("cons", [128, 2 + 3 * T], F32, isOutput=False)
    ident = nc.declare_dram_parameter("ident", [128, 128], BF16, isOutput=False)
    out = nc.declare_dram_parameter("out", [L, R], F32, isOutput=True)

    with TileContext(nc) as tc:
        with (
            tc.tile_pool(name="wpool", bufs=1) as wpool,
            tc.tile_pool(name="tepool", bufs=2) as tepool,
            tc.tile_pool(name="state", bufs=1) as state,
            tc.tile_pool(name="sppool", bufs=2) as sppool,
            tc.tile_pool(name="rpool", bufs=2) as rpool,
            tc.tile_pool(name="appool", bufs=2, space="PSUM") as appool,
        ):
            # ---- resident weights ----
            WA_CH = [2, 4, 8, 12]  # k-tiles per wallA chunk
            WA_OFF = [0, 2, 6, 14]
            wa_ck = [
                wpool.tile([128, WA_CH[c], F], FP8, tag=f"wallA{c}", name=f"wa{c}")
                for c in range(len(WA_CH))
            ]

            wallWb = wpool.tile([128, NK, F], FP8, tag="wallWb", name="wallWb")
            wallSe = wpool.tile([128, NK, TNB], FP8, tag="wallSe", name="wallSe")
            wallW1 = wpool.tile([128, NG, H], FP8, tag="wallW1", name="wallW1")
            wallW2 = wpool.tile([128, NG, LP], FP8, tag="wallW2", name="wallW2")
            cons_sb = wpool.tile([128, 2 + 3 * T], F32, tag="cons", name="cons_sb")
            ident_sb = wpool.tile([128, 128], BF16, tag="ident", name="ident_sb")

            def waT(jp, g):
                k = 2 * jp
                for c in range(len(WA_CH) - 1, -1, -1):
                    if k >= WA_OFF[c]:
                        kk = k - WA_OFF[c]
                        return wa_ck[c][:, kk : kk + 2, g * 128 : (g + 1) * 128]

            # ---- states ----
            M = state.tile([128, NG * R], BF16, tag="M", name="M")
            ML = state.tile([128, NG * R], BF16, tag="ML", name="ML")
            bs_sb = state.tile([128, NG, TNB], BF16, tag="bs", name="bs_sb")
            sp2s = [state.tile([128, NG, R], FP8, tag=f"sp2_{t}", name=f"sp2_{t}")
                    for t in range(T)]

            nc.vector.memset(M[:], 0.0)
            nc.gpsimd.memset(ML[:], 0.0)

            # ---- te DMA ----
            TE0_CH = [2, 4, 8, 12]   # k-tiles per pair-0 chunk
            TE0_OFF = [0, 2, 6, 14]

            def emit_ap0_chunk(psum, te_tiles, c):
                for kk in range(TE0_CH[c] // 2):
                    jp = TE0_OFF[c] // 2 + kk
                    for g in range(NG):
                        nc.tensor.matmul(
                            psum[:, g, :],
                            lhsT=waT(jp, g),
                            rhs=te_tiles[c][:, 2 * kk : 2 * kk + 2, :],
                            start=(jp == 0),
                            stop=False,
                            perf_mode=DR,
                        )

            def emit_ap_group(psum, te_tile, jp_lo, jp_hi, stop_at_last=False):
                for jp in range(jp_lo, jp_hi):
                    for g in range(NG):
                        nc.tensor.matmul(
                            psum[:, g, :],
                            lhsT=waT(jp, g),
                            rhs=te_tile[:, 2 * jp : 2 * jp + 2, :],
                            start=(jp == 0),
                            stop=(stop_at_last and jp == NKP - 1
                                  and g == NG - 1),
                            perf_mode=DR,
                        )

            def emit_ident(psum, pair, stop=True):
                # inject scaled cumulative basal, broadcast over S; when stop
                # is True it also closes the psum accumulation of each bank
                for g in range(NG):
                    rhs = (
                        bs_sb[:, g, :]
                        .rearrange("p (t b) -> p t b", b=NB)[:, 2 * pair : 2 * pair + 2, :]
                        .unsqueeze(3)
                        .broadcast_to([128, 2, NB, S])
                    )
                    nc.tensor.matmul(
                        psum[:, g, :],
                        lhsT=ident_sb[:],
                        rhs=rhs,
                        start=False,
                        stop=stop,
                    )

            # ---- per-substep emitters ----
            def emit_sub(psum, t):
                sub = t % 2
                th1 = float(2 ** (t + 1))
                src = psum[:, :, sub * R : (sub + 1) * R]
                # M = psum*2^(t-1)/1024 + M'  (psum holds scaled A_t + B_t)
                mstt = nc.vector.scalar_tensor_tensor(
                    M[:], src, float(2 ** (t - 1)) / SC_AP, M[:],
                    OP.mult, OP.add,
                )
                sp = sppool.tile([128, NG, R], FP8, tag="sp", name="sp")
                nc.vector.tensor_scalar(sp[:], M[:], th1, None, OP.is_gt)
                nc.vector.scalar_tensor_tensor(
                    M[:], M[:], th1, M[:], OP.is_le, OP.mult
                )
                return sp, mstt

            def emit_w1(psum, sp, t):
                sub = t % 2
                first = None
                for g in range(NG):
                    bank = 2 * sub + g // 2
                    dst = psum[:, bank, (g % 2) * R : (g % 2 + 1) * R]
                    for j in range(2):
                        mm = nc.tensor.matmul(
                            dst,
                            lhsT=wallW1[:, 2 * j : 2 * j + 2, g * 128 : (g + 1) * 128],
                            rhs=sp[:, 2 * j : 2 * j + 2, :],
                            start=(g % 2 == 0 and j == 0),
                            stop=(g % 2 == 1 and j == 1),
                            perf_mode=DR,
                        )
                        if first is None:
                            first = mm
                return first

            def emit_l2(psum, t):
                sub = t % 2
                th2 = float(2 ** t)
                hq = psum[:, 2 * sub : 2 * sub + 2, :].rearrange("p a b -> p (a b)")
                nc.vector.scalar_tensor_tensor(
                    ML[:], hq, float(2 ** t) / SC_W, ML[:], OP.mult, OP.add
                )
                nc.vector.tensor_scalar(sp2s[t][:], ML[:], th2, None, OP.is_gt)
                nc.vector.scalar_tensor_tensor(
                    ML[:], ML[:], th2, ML[:], OP.is_le, OP.mult
                )

            # ================= schedule =================
            # One serialized DMA chain delivering bytes in PE consumption
            # order: te0/wallA chunks interleaved, basal weights early.
            te0_tiles = [
                tepool.tile([128, TE0_CH[c], 2 * R], FP8, tag=f"te0_{c}",
                            name=f"te0_{c}")
                for c in range(len(TE0_CH))
            ]

            def te0_dma(c):
                return nc.sync.dma_start(
                    te0_tiles[c][:],
                    teT[0][:, TE0_OFF[c] * 2 * R
                           : (TE0_OFF[c] + TE0_CH[c]) * 2 * R],
                )

            def wa_dma(c):
                return nc.sync.dma_start(
                    wa_ck[c][:],
                    wallA[:, WA_OFF[c] * F : (WA_OFF[c] + WA_CH[c]) * F],
                )

            # Sync executes triggers in emission order, so emit them in PE
            # need-order; per-chain deps keep each stream serialized.
            chain_te, chain_wa, chain_wb = [], [], []
            chain_te.append(te0_dma(0))
            chain_wa.append(wa_dma(0))
            chain_te.append(te0_dma(1))
            chain_wa.append(wa_dma(1))
            chain_wb.append(nc.sync.dma_start(wallWb[:], wallM[:, : NK * F]))
            chain_wb.append(nc.sync.dma_start(wallSe[:], wallM[:, NK * F :]))
            chain_te.append(te0_dma(2))
            chain_wa.append(wa_dma(2))
            chain_te.append(te0_dma(3))
            chain_wa.append(wa_dma(3))
            chain_wb.append(nc.sync.dma_start(wallW1[:], wallB[:, : NG * H]))
            chain_wb.append(nc.sync.dma_start(wallW2[:], wallB[:, NG * H :]))
            chain_wb.append(nc.sync.dma_start(cons_sb[:], cons[:]))
            chain_wb.append(nc.sync.dma_start(ident_sb[:], ident[:]))
            for a, b in zip(chain_wb, chain_wb[1:]):
                add_dep_helper(b.ins, a.ins, reason="prologue dma chain")

            ap_cur = appool.tile([128, NG, 2 * R], F32, tag="ap", name="ap0")
            ap_nxt = appool.tile([128, NG, 2 * R], F32, tag="ap", name="ap1")

            def emit_basal(j_lo, j_hi):
                # basal matmuls into ap_nxt banks (consumed+copied before
                # pair1 apical start=True re-zeroes them); columns hold the
                # scaled cumulative basal per substep (host presums se)
                for g in range(NG):
                    for j in range(j_lo, j_hi):
                        nc.tensor.matmul(
                            ap_nxt[:, g, :TNB],
                            lhsT=wallWb[:, 2 * j : 2 * j + 2,
                                        g * 128 : (g + 1) * 128],
                            rhs=wallSe[:, 2 * j : 2 * j + 2, :],
                            start=(j == 0),
                            stop=(j == NKP - 1),
                            perf_mode=DR,
                        )

            emit_ap0_chunk(ap_cur, te0_tiles, 0)
            emit_ap0_chunk(ap_cur, te0_tiles, 1)
            emit_ap0_chunk(ap_cur, te0_tiles, 2)
            emit_basal(0, NKP)
            emit_ap0_chunk(ap_cur, te0_tiles, 3)
            nc.vector.tensor_copy(bs_sb[:], ap_nxt[:, :, :TNB])
            emit_ident(ap_cur, 0)


            # main loop
            def emit_w2(t):
                # o accumulates in a dead bank of the pair-2 psum tile
                for j in range(2):
                    nc.tensor.matmul(
                        o_ps,
                        lhsT=wallW2[:, 2 * j : 2 * j + 2, :],
                        rhs=sp2s[t][:, 2 * j : 2 * j + 2, :],
                        start=(t == 0 and j == 0),
                        stop=(t == T - 1 and j == 1),
                        perf_mode=DR,
                    )

            teP = None
            o_ps = None
            for pair in range(NPAIR):
                t0, t1 = 2 * pair, 2 * pair + 1
                last = pair + 1 == NPAIR
                if not last:
                    teP = tepool.tile([128, NK, 2 * R], FP8, tag="teP",
                                      name=f"teP{pair + 1}")
                    for lo, hi in ((0, 10), (10, 18), (18, NK)):
                        nc.sync.dma_start(
                            teP[:, lo:hi, :],
                            teT[pair + 1][:, lo * 2 * R : hi * 2 * R],
                        )
                sp0, mstt0 = emit_sub(ap_cur, t0)
                sp1, mstt1 = emit_sub(ap_cur, t1)
                if not last:
                    emit_ap_group(ap_nxt, teP, 0, 7)
                    emit_ident(ap_nxt, pair + 1, stop=False)
                else:
                    for t in range(0, 3):
                        emit_w2(t)
                w1_first = emit_w1(ap_cur, sp0, t0)
                # W1(t0)'s start=True zeroes whole banks incl. the halves
                # Mstt(t1) reads; no AP overlap so force the order explicitly
                add_dep_helper(w1_first.ins, mstt1.ins,
                               reason="hq bank zero after Mstt t1")
                emit_l2(ap_cur, t0)
                if not last:
                    emit_ap_group(ap_nxt, teP, 7, 11)
                else:
                    for t in range(3, 6):
                        emit_w2(t)
                emit_w1(ap_cur, sp1, t1)
                emit_l2(ap_cur, t1)
                if not last:
                    emit_ap_group(ap_nxt, teP, 11, NKP, stop_at_last=True)
                if pair + 2 < NPAIR:
                    ap_cur, ap_nxt = ap_nxt, appool.tile(
                        [128, NG, 2 * R], F32, tag="ap", name=f"ap{pair + 2}"
                    )
                elif pair + 2 == NPAIR:
                    # ap_cur (pair-2 tile) retires after this pair; bank 0
                    # hosts the W2 output accumulation
                    o_ps = ap_cur[0:LP, 0, 0:R]
                    ap_cur, ap_nxt = ap_nxt, ap_cur

            emit_w2(6)
            emit_w2(7)
            out_sb = state.tile([L, R], F32, tag="out_sb", name="out_sb")
            nc.scalar.activation(
                out_sb[:], ap_nxt[0:L, 0, 0:R], AF.Identity,
                bias=cons_sb[0:L, 0:1], scale=1.0 / (T * SC_W),
            )
            nc.sync.dma_start(out[:], out_sb[:])

    return nc


# ---------------- host side ----------------

def _padk(a, kd=KD):
    o = np.zeros((kd,) + a.shape[1:], a.dtype)
    o[: a.shape[0]] = a
    return o


def _kmaj(a, cols):
    """[KD, cols] f32 -> [128, NK*cols] fp8, [p, k*cols+c] = a[k*128+p, c]"""
    e4 = ml_dtypes.float8_e4m3
    nk = a.shape[0] // 128
    return np.ascontiguousarray(
        a.reshape(nk, 128, cols).transpose(1, 0, 2).reshape(128, nk * cols)
    ).astype(e4)


def prep_in_maps(inputs):
    se = np.asarray(inputs["state_embedding"], np.float32)
    te = np.asarray(inputs["tau_embedding"], np.float32)
    Wb = np.asarray(inputs["Wb"], np.float32)
    Wa = np.asarray(inputs["Wa"], np.float32)
    W1 = np.asarray(inputs["W1"], np.float32)
    b1 = np.asarray(inputs["b1"], np.float32)
    W2 = np.asarray(inputs["W2"], np.float32)
    b2 = np.asarray(inputs["b2"], np.float32)
    e4 = ml_dtypes.float8_e4m3

    wallA_np = _kmaj(_padk(Wa.T * SC_W), F)
    wallWb_np = _kmaj(_padk(Wb.T * SC_W), F)
    wallM_np = np.empty((128, NK * F + NK * TNB), e4)
    wallM_np[:, : NK * F] = wallWb_np

    wallB_np = np.empty((128, NG * H + NG * LP), e4)
    wallB_np[:, : NG * H] = _kmaj(np.ascontiguousarray(W1.T) * SC_W, H)
    w2p = np.zeros((H, LP), np.float32)
    w2p[:, :L] = W2.T * SC_W
    wallB_np[:, NG * H :] = _kmaj(w2p, LP)

    # fold the A/B cumulative recurrences into the matmul columns:
    # column for substep t carries 16 * sum_{tau<=t} 2^(tau-t) * x_tau
    wts = (2.0 ** np.arange(T)).astype(np.float32)[:, None, None]
    te = np.cumsum(te * wts, axis=0) / wts
    se = np.cumsum(se * wts, axis=0) / wts

    ident_np = np.eye(128, dtype=ml_dtypes.bfloat16)

    cons_np = np.zeros((128, 2 + 3 * T), np.float32)
    cons_np[:L, 0] = b2
    for t in range(T):
        cons_np[:, 2 + t] = -(2.0 ** t)          # relu bias
        cons_np[:, 2 + T + t] = 2.0 ** (t + 1)   # layer-1 spike threshold
        cons_np[:, 2 + 2 * T + t] = 2.0 ** t     # LIF reset threshold
    # b1 is zero in this problem; assert so silent wrongness can't slip in
    assert not np.any(b1), "kernel assumes b1 == 0"

    in_maps = []
    for i in range(N_CORES):
        tei = te[:, i * R : (i + 1) * R, :]                  # [T, R, DT]
        tei = (tei * SC_IN).reshape(NPAIR, 2, R, DT)
        tei_p = np.zeros((NPAIR, 2, R, KD), np.float32)
        tei_p[..., :DT] = tei
        teT_np = np.ascontiguousarray(
            tei_p.reshape(NPAIR, 2, R, NK, 128)
            .transpose(0, 4, 3, 1, 2)                        # [pair,p,k,sub,r]
            .reshape(NPAIR, 128, NK * 2 * R)
        ).astype(e4)

        sei = se[:, i * NB : (i + 1) * NB, :]                # [T, NB, DS]
        seT = _padk(np.ascontiguousarray(sei.reshape(TNB, DS).T) * SC_IN)
        wallM_i = wallM_np.copy()
        wallM_i[:, NK * F :] = _kmaj(seT, TNB)
        in_maps.append(dict(teT=teT_np, wallA=wallA_np, wallM=wallM_i,
                            wallB=wallB_np, cons=cons_np, ident=ident_np))
    return in_maps


def assemble_out(core_outs):
    full = np.stack([np.asarray(o, np.float32) for o in core_outs], axis=0)
    full = full.reshape(N_CORES, L, NB, S).transpose(0, 2, 1, 3)
    return np.ascontiguousarray(full.reshape(B, L, S))


_NC_CACHE = {}


def get_nc():
    if "nc" not in _NC_CACHE:
        last = None
        for _ in range(6):
            try:
                _NC_CACHE["nc"] = build_nc()
                break
            except Exception as e:  # rare scheduler-order race-detector trip
                last = e
        else:
            raise last
    return _NC_CACHE["nc"]


def run_sharded(in_maps, trace=False, **kw):
    nc = get_nc()
    if not getattr(nc, "_waits_split", False):
        _split_excess_waits(nc)
        nc._waits_split = True
    return run_bass_kernel_spmd(
        nc, in_maps, core_ids=list(range(N_CORES)), trace=trace, **kw
    )


def kernel(**inputs):
    in_maps = prep_in_maps(inputs)
    res = run_sharded(in_maps)
    return assemble_out([res.results[i]["out"] for i in range(N_CORES)])


# revision 4
# speedup vs baseline: 1.0612x; 1.0394x over previous
"""Trainium2 Bass kernel for nn_MCQuantiles (ThreeCompNode SNN scan) — v3.

Strategy:
- All heavy matmuls in fp8 e4m3 with DoubleRow perf mode (2 k-tiles per
  instruction = 2x effective PE throughput vs bf16). K padded 3136->3328
  (26 k-tiles = 13 DoubleRow pairs).
- The apical membrane's cumulative sum is folded into the matmul: host
  sends, for substep t, the column 16*sum_{tau<=t} 2^(tau-t)*te_tau, so
  PSUM holds 1024*2^-(t-1)*A_t directly. The basal cumulative term (same
  host presum over se) is injected into the same PSUM banks by a [128x128]
  identity matmul with an S-broadcast rhs. Layer-1 per step is then just
  one stt from PSUM (M = psum*2^(t-1)/1024 + M'), one is_gt -> fp8 spike,
  one is_le*mult reset, all merged [128, 1024] across the 4 f-groups.
- LIF layer: stt from hq PSUM, is_gt -> fp8 sp2 (stored per t), is_le*mult
  reset. W2 runs as a 16-instruction tail over the stored sp2 tiles.
- PSUM: 2 rotating [128, NG, 2R] fp32 tiles (4 banks each = all 8 banks).
  hq and W2 outputs are carved into banks of the pair tile whose apical
  data is already consumed (the two M-updates of a pair run before any hq
  matmul so start=True bank zeroing never kills live data).
"""
import numpy as np
import ml_dtypes

import bass_rust
import concourse.bass as bass
import concourse.mybir as mybir
from concourse.bass_utils import run_bass_kernel_spmd
from concourse.tile import TileContext
from concourse.tile_rust import add_dep_helper

# ----- problem constants (hardcoded per contract) -----
T, B, S = 8, 64, 32
DS = DT = 3136
F = H = 512
L = 18
N_CORES = 8
NB = B // N_CORES              # 8 batches per core
R = NB * S                     # 256 rows per core
NK = 26                        # k-tiles of 128 (3136 padded to 3328)
KD = NK * 128
NKP = NK // 2                  # 13 DoubleRow k-pairs
NPAIR = T // 2                 # 4 time-step pairs
NG = F // 128                  # 4 feature groups
TNB = T * NB                   # 64 basal columns
LP = 32                        # L padded for DoubleRow lhsT stride alignment

CH_K = [2, 4, 4, 4, 6, 6]      # k-tiles per DMA chunk (26 total, pair-aligned)
CH_OFF = [0, 2, 6, 10, 14, 20]
NCHUNK = len(CH_K)

F32 = mybir.dt.float32
BF16 = mybir.dt.bfloat16
FP8 = mybir.dt.float8e4
OP = mybir.AluOpType
DR = mybir.MatmulPerfMode.DoubleRow
AF = mybir.ActivationFunctionType

SC_IN = 16.0                   # activation host scale
SC_W = 64.0                    # weight host scale
SC_AP = SC_IN * SC_W           # apical/basal psum carry 1024x


def _patch_tile_drain():
    """This walrus build allows a single sync-wait per TPB_CTRL Drain; Tile's
    kernel-tail drain attaches one wait per active logical proc. Split them
    across a chain of drains."""
    def _patched(self, tick_clock, wait_clock):
        nc = self.nc
        drain_inst = nc.sync.drain()
        wait_clock.add_sem_waits(
            drain_inst.ins, bass_rust.ScopedClock({None: tick_clock.global_clock})
        )
        si = drain_inst.ins.sync_info
        if si is not None and len(si.on_wait) > 1:
            waits = list(si.on_wait)
            drain_inst.ins.sync_info = mybir.SyncInfo(
                on_wait=waits[:1], on_update=list(si.on_update)
            )
            for w in waits[1:]:
                extra = nc.sync.drain()
                extra.ins.sync_info = mybir.SyncInfo(on_wait=[w], on_update=[])
        nc.all_engine_barrier()
        popped = nc._tile_sem_poison_stack.pop()
        assert popped is self._sem_poison
        nc.clear_and_free_semaphores(list(self.sems.allocated().values()))
        nc.all_engine_barrier()

    TileContext._drain_and_barrier = _patched


def _split_excess_waits(nc, limit=1):
    """Walrus here rejects instructions carrying more than ~1 sync-wait. Move
    excess waits onto same-engine NoOps inserted just before the instruction."""
    for fn in nc.m.functions:
        for bb in fn.blocks:
            new = []
            changed = False
            for inst in bb.instructions:
                si = getattr(inst, "sync_info", None)
                ow = list(si.on_wait) if si is not None and si.on_wait else []
                if len(ow) > limit:
                    extra = ow[limit:]
                    for j in range(0, len(extra), limit):
                        nop = mybir.InstNoOp(
                            name=f"{inst.name}-ws{j}", ins=[], outs=[]
                        )
                        nop.engine = inst.engine
                        nop.sync_info = mybir.SyncInfo(
                            on_wait=extra[j : j + limit], on_update=[]
                        )
                        new.append(nop)
                    inst.sync_info = mybir.SyncInfo(
                        on_wait=ow[:limit], on_update=list(si.on_update)
                    )
                    changed = True
                new.append(inst)
            if changed:
                try:
                    bb.instructions[:] = new
                except TypeError:
                    bb.instructions = new


def build_nc():
    _patch_tile_drain()
    nc = bass.Bass()

    teT = nc.declare_dram_parameter("teT", [NPAIR, 128, NK * 2 * R], FP8, isOutput=False)
    wallA = nc.declare_dram_parameter("wallA", [128, NK * F], FP8, isOutput=False)
    wallM = nc.declare_dram_parameter("wallM", [128, NK * F + NK * TNB], FP8, isOutput=False)
    wallB = nc.declare_dram_parameter("wallB", [128, NG * H + NG * LP], FP8, isOutput=False)
    cons = nc.declare_dram_parameter("cons", [128, 2 + 3 * T], F32, isOutput=False)
    ident = nc.declare_dram_parameter("ident", [128, 128], BF16, isOutput=False)
    out = nc.declare_dram_parameter("out", [L, R], F32, isOutput=True)

    with TileContext(nc) as tc:
        with (
            tc.tile_pool(name="wpool", bufs=1) as wpool,
            tc.tile_pool(name="tepool", bufs=2) as tepool,
            tc.tile_pool(name="state", bufs=1) as state,
            tc.tile_pool(name="sppool", bufs=2) as sppool,
            tc.tile_pool(name="rpool", bufs=2) as rpool,
            tc.tile_pool(name="appool", bufs=2, space="PSUM") as appool,
        ):
            # ---- resident weights ----
            WA_CH = [2, 4, 8, 12]  # k-tiles per wallA chunk
            WA_OFF = [0, 2, 6, 14]
            wa_ck = [
                wpool.tile([128, WA_CH[c], F], FP8, tag=f"wallA{c}", name=f"wa{c}")
                for c in range(len(WA_CH))
            ]

            wallWb = wpool.tile([128, NK, F], FP8, tag="wallWb", name="wallWb")
            wallSe = wpool.tile([128, NK, TNB], FP8, tag="wallSe", name="wallSe")
            wallW1 = wpool.tile([128, NG, H], FP8, tag="wallW1", name="wallW1")
            wallW2 = wpool.tile([128, NG, LP], FP8, tag="wallW2", name="wallW2")
            cons_sb = wpool.tile([128, 2 + 3 * T], F32, tag="cons", name="cons_sb")
            ident_sb = wpool.tile([128, 128], BF16, tag="ident", name="ident_sb")

            def waT(jp, g):
                k = 2 * jp
                for c in range(len(WA_CH) - 1, -1, -1):
                    if k >= WA_OFF[c]:
                        kk = k - WA_OFF[c]
                        return wa_ck[c][:, kk : kk + 2, g * 128 : (g + 1) * 128]

            # ---- states ----
            M = state.tile([128, NG * R], BF16, tag="M", name="M")
            ML = state.tile([128, NG * R], BF16, tag="ML", name="ML")
            bs_sb = state.tile([128, NG, TNB], BF16, tag="bs", name="bs_sb")
            sp2s = [state.tile([128, NG, R], FP8, tag=f"sp2_{t}", name=f"sp2_{t}")
                    for t in range(T)]

            nc.vector.memset(M[:], 0.0)
            nc.gpsimd.memset(ML[:], 0.0)

            # ---- te DMA ----
            TE0_CH = [2, 4, 8, 12]   # k-tiles per pair-0 chunk
            TE0_OFF = [0, 2, 6, 14]

            def emit_ap0_chunk(psum, te_tiles, c):
                for kk in range(TE0_CH[c] // 2):
                    jp = TE0_OFF[c] // 2 + kk
                    for g in range(NG):
                        nc.tensor.matmul(
                            psum[:, g, :],
                            lhsT=waT(jp, g),
                            rhs=te_tiles[c][:, 2 * kk : 2 * kk + 2, :],
                            start=(jp == 0),
                            stop=False,
                            perf_mode=DR,
                        )

            def emit_ap_group(psum, te_tile, jp_lo, jp_hi, stop_at_last=False):
                for jp in range(jp_lo, jp_hi):
                    for g in range(NG):
                        nc.tensor.matmul(
                            psum[:, g, :],
                            lhsT=waT(jp, g),
                            rhs=te_tile[:, 2 * jp : 2 * jp + 2, :],
                            start=(jp == 0),
                            stop=(stop_at_last and jp == NKP - 1
                                  and g == NG - 1),
                            perf_mode=DR,
                        )

            def emit_ident(psum, pair, stop=True):
                # inject scaled cumulative basal, broadcast over S; when stop
                # is True it also closes the psum accumulation of each bank
                for g in range(NG):
                    rhs = (
                        bs_sb[:, g, :]
                        .rearrange("p (t b) -> p t b", b=NB)[:, 2 * pair : 2 * pair + 2, :]
                        .unsqueeze(3)
                        .broadcast_to([128, 2, NB, S])
                    )
                    nc.tensor.matmul(
                        psum[:, g, :],
                        lhsT=ident_sb[:],
                        rhs=rhs,
                        start=False,
                        stop=stop,
                    )

            # ---- per-substep emitters ----
            def emit_sub(psum, t):
                sub = t % 2
                th1 = float(2 ** (t + 1))
                src = psum[:, :, sub * R : (sub + 1) * R]
                # M = psum*2^(t-1)/1024 + M'  (psum holds scaled A_t + B_t)
                mstt = nc.vector.scalar_tensor_tensor(
                    M[:], src, float(2 ** (t - 1)) / SC_AP, M[:],
                    OP.mult, OP.add,
                )
                sp = sppool.tile([128, NG, R], FP8, tag="sp", name="sp")
                nc.vector.tensor_scalar(sp[:], M[:], th1, None, OP.is_gt)
                nc.vector.scalar_tensor_tensor(
                    M[:], M[:], th1, M[:], OP.is_le, OP.mult
                )
                return sp, mstt

            def emit_w1(psum, sp, t):
                sub = t % 2
                first = None
                for g in range(NG):
                    bank = 2 * sub + g // 2
                    dst = psum[:, bank, (g % 2) * R : (g % 2 + 1) * R]
                    for j in range(2):
                        mm = nc.tensor.matmul(
                            dst,
                            lhsT=wallW1[:, 2 * j : 2 * j + 2, g * 128 : (g + 1) * 128],
                            rhs=sp[:, 2 * j : 2 * j + 2, :],
                            start=(g % 2 == 0 and j == 0),
                            stop=(g % 2 == 1 and j == 1),
                            perf_mode=DR,
                        )
                        if first is None:
                            first = mm
                return first

            def emit_l2a(psum, t):
                # LIF integrate from hq PSUM; prompt (releases the psum tile)
                sub = t % 2
                hq = psum[:, 2 * sub : 2 * sub + 2, :].rearrange("p a b -> p (a b)")
                nc.vector.scalar_tensor_tensor(
                    ML[:], hq, float(2 ** t) / SC_W, ML[:], OP.mult, OP.add
                )

            def emit_l2b(t):
                # LIF spike + reset; deferrable until before the next MLstt
                th2 = float(2 ** t)
                nc.vector.tensor_scalar(sp2s[t][:], ML[:], th2, None, OP.is_gt)
                nc.vector.scalar_tensor_tensor(
                    ML[:], ML[:], th2, ML[:], OP.is_le, OP.mult
                )

            # ================= schedule =================
            # One serialized DMA chain delivering bytes in PE consumption
            # order: te0/wallA chunks interleaved, basal weights early.
            te0_tiles = [
                tepool.tile([128, TE0_CH[c], 2 * R], FP8, tag=f"te0_{c}",
                            name=f"te0_{c}")
                for c in range(len(TE0_CH))
            ]

            def te0_dma(c):
                return nc.sync.dma_start(
                    te0_tiles[c][:],
                    teT[0][:, TE0_OFF[c] * 2 * R
                           : (TE0_OFF[c] + TE0_CH[c]) * 2 * R],
                )

            def wa_dma(c):
                return nc.sync.dma_start(
                    wa_ck[c][:],
                    wallA[:, WA_OFF[c] * F : (WA_OFF[c] + WA_CH[c]) * F],
                )

            # Sync executes triggers in emission order, so emit them in PE
            # need-order; per-chain deps keep each stream serialized.
            chain_te, chain_wa, chain_wb = [], [], []
            chain_te.append(te0_dma(0))
            chain_wa.append(wa_dma(0))
            chain_te.append(te0_dma(1))
            chain_wa.append(wa_dma(1))
            chain_wb.append(nc.sync.dma_start(wallWb[:], wallM[:, : NK * F]))
            chain_wb.append(nc.sync.dma_start(wallSe[:], wallM[:, NK * F :]))
            chain_te.append(te0_dma(2))
            chain_wa.append(wa_dma(2))
            chain_te.append(te0_dma(3))
            chain_wa.append(wa_dma(3))
            chain_wb.append(nc.sync.dma_start(wallW1[:], wallB[:, : NG * H]))
            chain_wb.append(nc.sync.dma_start(wallW2[:], wallB[:, NG * H :]))
            chain_wb.append(nc.sync.dma_start(cons_sb[:], cons[:]))
            chain_wb.append(nc.sync.dma_start(ident_sb[:], ident[:]))
            for a, b in zip(chain_wb, chain_wb[1:]):
                add_dep_helper(b.ins, a.ins, reason="prologue dma chain")

            ap_cur = appool.tile([128, NG, 2 * R], F32, tag="ap", name="ap0")
            ap_nxt = appool.tile([128, NG, 2 * R], F32, tag="ap", name="ap1")

            def emit_basal(j_lo, j_hi):
                # basal matmuls into ap_nxt banks (consumed+copied before
                # pair1 apical start=True re-zeroes them); columns hold the
                # scaled cumulative basal per substep (host presums se)
                for g in range(NG):
                    for j in range(j_lo, j_hi):
                        nc.tensor.matmul(
                            ap_nxt[:, g, :TNB],
                            lhsT=wallWb[:, 2 * j : 2 * j + 2,
                                        g * 128 : (g + 1) * 128],
                            rhs=wallSe[:, 2 * j : 2 * j + 2, :],
                            start=(j == 0),
                            stop=(j == NKP - 1),
                            perf_mode=DR,
                        )

            emit_ap0_chunk(ap_cur, te0_tiles, 0)
            emit_ap0_chunk(ap_cur, te0_tiles, 1)
            emit_ap0_chunk(ap_cur, te0_tiles, 2)
            emit_basal(0, NKP)
            emit_ap0_chunk(ap_cur, te0_tiles, 3)
            nc.vector.tensor_copy(bs_sb[:], ap_nxt[:, :, :TNB])
            emit_ident(ap_cur, 0)


            # main loop
            def emit_w2(t):
                # o accumulates in a dead bank of the pair-2 psum tile
                for j in range(2):
                    nc.tensor.matmul(
                        o_ps,
                        lhsT=wallW2[:, 2 * j : 2 * j + 2, :],
                        rhs=sp2s[t][:, 2 * j : 2 * j + 2, :],
                        start=(t == 0 and j == 0),
                        stop=(t == T - 1 and j == 1),
                        perf_mode=DR,
                    )

            teP = None
            o_ps = None
            pending_l2b = None
            for pair in range(NPAIR):
                t0, t1 = 2 * pair, 2 * pair + 1
                last = pair + 1 == NPAIR
                if not last:
                    teP = tepool.tile([128, NK, 2 * R], FP8, tag="teP",
                                      name=f"teP{pair + 1}")
                    for lo, hi in ((0, 10), (10, 18), (18, NK)):
                        nc.sync.dma_start(
                            teP[:, lo:hi, :],
                            teT[pair + 1][:, lo * 2 * R : hi * 2 * R],
                        )
                sp0, mstt0 = emit_sub(ap_cur, t0)
                sp1, mstt1 = emit_sub(ap_cur, t1)
                if pending_l2b is not None:
                    emit_l2b(pending_l2b)
                    pending_l2b = None
                if not last:
                    emit_ap_group(ap_nxt, teP, 0, 7)
                    emit_ident(ap_nxt, pair + 1, stop=False)
                else:
                    for t in range(0, 3):
                        emit_w2(t)
                w1_first = emit_w1(ap_cur, sp0, t0)
                # W1(t0)'s start=True zeroes whole banks incl. the halves
                # Mstt(t1) reads; no AP overlap so force the order explicitly
                add_dep_helper(w1_first.ins, mstt1.ins,
                               reason="hq bank zero after Mstt t1")
                emit_l2a(ap_cur, t0)
                emit_l2b(t0)
                if not last:
                    emit_ap_group(ap_nxt, teP, 7, 11)
                else:
                    for t in range(3, 6):
                        emit_w2(t)
                emit_w1(ap_cur, sp1, t1)
                emit_l2a(ap_cur, t1)
                pending_l2b = t1
                if not last:
                    emit_ap_group(ap_nxt, teP, 11, NKP, stop_at_last=True)
                if pair + 2 < NPAIR:
                    ap_cur, ap_nxt = ap_nxt, appool.tile(
                        [128, NG, 2 * R], F32, tag="ap", name=f"ap{pair + 2}"
                    )
                elif pair + 2 == NPAIR:
                    # ap_cur (pair-2 tile) retires after this pair; bank 0
                    # hosts the W2 output accumulation
                    o_ps = ap_cur[0:LP, 0, 0:R]
                    ap_cur, ap_nxt = ap_nxt, ap_cur

            emit_l2b(7)
            emit_w2(6)
            emit_w2(7)
            out_sb = state.tile([L, R], F32, tag="out_sb", name="out_sb")
            nc.scalar.activation(
                out_sb[:], ap_nxt[0:L, 0, 0:R], AF.Identity,
                bias=cons_sb[0:L, 0:1], scale=1.0 / (T * SC_W),
            )
            nc.sync.dma_start(out[:], out_sb[:])

    return nc


# ---------------- host side ----------------

def _padk(a, kd=KD):
    o = np.zeros((kd,) + a.shape[1:], a.dtype)
    o[: a.shape[0]] = a
    return o


def _kmaj(a, cols):
    """[KD, cols] f32 -> [128, NK*cols] fp8, [p, k*cols+c] = a[k*128+p, c]"""
    e4 = ml_dtypes.float8_e4m3
    nk = a.shape[0] // 128
    return np.ascontiguousarray(
        a.reshape(nk, 128, cols).transpose(1, 0, 2).reshape(128, nk * cols)
    ).astype(e4)


def prep_in_maps(inputs):
    se = np.asarray(inputs["state_embedding"], np.float32)
    te = np.asarray(inputs["tau_embedding"], np.float32)
    Wb = np.asarray(inputs["Wb"], np.float32)
    Wa = np.asarray(inputs["Wa"], np.float32)
    W1 = np.asarray(inputs["W1"], np.float32)
    b1 = np.asarray(inputs["b1"], np.float32)
    W2 = np.asarray(inputs["W2"], np.float32)
    b2 = np.asarray(inputs["b2"], np.float32)
    e4 = ml_dtypes.float8_e4m3

    wallA_np = _kmaj(_padk(Wa.T * SC_W), F)
    wallWb_np = _kmaj(_padk(Wb.T * SC_W), F)
    wallM_np = np.empty((128, NK * F + NK * TNB), e4)
    wallM_np[:, : NK * F] = wallWb_np

    wallB_np = np.empty((128, NG * H + NG * LP), e4)
    wallB_np[:, : NG * H] = _kmaj(np.ascontiguousarray(W1.T) * SC_W, H)
    w2p = np.zeros((H, LP), np.float32)
    w2p[:, :L] = W2.T * SC_W
    wallB_np[:, NG * H :] = _kmaj(w2p, LP)

    # fold the A/B cumulative recurrences into the matmul columns:
    # column for substep t carries 16 * sum_{tau<=t} 2^(tau-t) * x_tau
    wts = (2.0 ** np.arange(T)).astype(np.float32)[:, None, None]
    te = np.cumsum(te * wts, axis=0) / wts
    se = np.cumsum(se * wts, axis=0) / wts

    ident_np = np.eye(128, dtype=ml_dtypes.bfloat16)

    cons_np = np.zeros((128, 2 + 3 * T), np.float32)
    cons_np[:L, 0] = b2
    for t in range(T):
        cons_np[:, 2 + t] = -(2.0 ** t)          # relu bias
        cons_np[:, 2 + T + t] = 2.0 ** (t + 1)   # layer-1 spike threshold
        cons_np[:, 2 + 2 * T + t] = 2.0 ** t     # LIF reset threshold
    # b1 is zero in this problem; assert so silent wrongness can't slip in
    assert not np.any(b1), "kernel assumes b1 == 0"

    in_maps = []
    for i in range(N_CORES):
        tei = te[:, i * R : (i + 1) * R, :]                  # [T, R, DT]
        tei = (tei * SC_IN).reshape(NPAIR, 2, R, DT)
        tei_p = np.zeros((NPAIR, 2, R, KD), np.float32)
        tei_p[..., :DT] = tei
        teT_np = np.ascontiguousarray(
            tei_p.reshape(NPAIR, 2, R, NK, 128)
            .transpose(0, 4, 3, 1, 2)                        # [pair,p,k,sub,r]
            .reshape(NPAIR, 128, NK * 2 * R)
        ).astype(e4)

        sei = se[:, i * NB : (i + 1) * NB, :]                # [T, NB, DS]
        seT = _padk(np.ascontiguousarray(sei.reshape(TNB, DS).T) * SC_IN)
        wallM_i = wallM_np.copy()
        wallM_i[:, NK * F :] = _kmaj(seT, TNB)
        in_maps.append(dict(teT=teT_np, wallA=wallA_np, wallM=wallM_i,
                            wallB=wallB_np, cons=cons_np, ident=ident_np))
    return in_maps


def assemble_out(core_outs):
    full = np.stack([np.asarray(o, np.float32) for o in core_outs], axis=0)
    full = full.reshape(N_CORES, L, NB, S).transpose(0, 2, 1, 3)
    return np.ascontiguousarray(full.reshape(B, L, S))


_NC_CACHE = {}


def get_nc():
    if "nc" not in _NC_CACHE:
        last = None
        for _ in range(6):
            try:
                _NC_CACHE["nc"] = build_nc()
                break
            except Exception as e:  # rare scheduler-order race-detector trip
                last = e
        else:
            raise last
    return _NC_CACHE["nc"]


def run_sharded(in_maps, trace=False, **kw):
    nc = get_nc()
    if not getattr(nc, "_waits_split", False):
        _split_excess_waits(nc)
        nc._waits_split = True
    return run_bass_kernel_spmd(
        nc, in_maps, core_ids=list(range(N_CORES)), trace=trace, **kw
    )


def kernel(**inputs):
    in_maps = prep_in_maps(inputs)
    res = run_sharded(in_maps)
    return assemble_out([res.results[i]["out"] for i in range(N_CORES)])
